# revision 15
# baseline (speedup 1.0000x reference)
"""Dependency-GCN via host pre-gather + per-window PSUM accumulation
for 8 Trainium2 NeuronCores.  No scatter, no SWDGE, no collectives.

Strategy (single SPMD program):
  - Each core owns a contiguous range of 3750 destination nodes; edges
    are routed to their dst-owner core (fwd: dep, rev: gov).
  - Host pre-combines edges sharing (direction, relation, dst): their
    source rows are summed on the host, so each (direction, relation)
    group has at most ONE cell per dst.
  - Destinations are grouped into 30 windows of 128.  For window w and
    relation-weight r (20 edge rels + self as rel 20), a 128-column
    lhsT block holds the cell source features at column = dst % 128
    (zero columns where the (r, dst) cell is absent).  The 21 rel
    blocks of a window accumulate into ONE PSUM tile via matmul
    accumulation -- the "scatter" happens positionally in PSUM.
  - Bias rides as a k=21 matmul per window: lhsT = per-dst edge counts
    for each rel (+ const-1 row), rhs = [b_fwd; b_rev; b_self].  This
    reproduces out += cnt_r * b_r exactly (multi-edge cells carry
    their edge count).
  - The gather is done ON THE HOST: x_blocks [128, nblk*256] fp16
    holds, for block b, the transposed source features laid out as
    (k-partition, b*256 + k_half*128 + column) so plain contiguous
    DMA loads (multi-KB descriptor runs) yield ready-to-use matmul
    lhsT tiles.
  - Per window: 43 fp16 matmuls (FWL hides weight loads) -> one
    PSUM->SBUF fp32->fp16 copy (alternating Activation/DVE) -> one
    plain contiguous DMA write of the finished 128 output rows.
"""

import sys

if "/opt/trn_rl_repo" not in sys.path:
    sys.path.insert(0, "/opt/trn_rl_repo")

import os as _os
import numpy as np

import concourse.bacc as bacc
import concourse.mybir as mybir
from concourse.tile import TileContext
from concourse.bass_utils import run_bass_kernel_spmd

F32 = mybir.dt.float32
F16 = mybir.dt.float16

N_NODES = 30000
N_REL = 10
D = 256
N_CORES = 8
NODES_PER_CORE = N_NODES // N_CORES          # 3750
NW = (NODES_PER_CORE + 127) // 128            # 30 windows of 128 dsts
NRW = 21                                      # 20 edge rels + self
GBC = int(_os.environ.get("GCN_GBC", "8"))   # blocks per load chunk


# ---------------------------------------------------------------- host prep

def prepare(x, W_self, b_self, W_fwd, b_fwd, W_rev, b_rev,
            dep_idx, rel_idx, gov_idx):
    dep_idx = np.asarray(dep_idx).astype(np.int64)
    rel_idx = np.asarray(rel_idx).astype(np.int64)
    gov_idx = np.asarray(gov_idx).astype(np.int64)
    x = np.asarray(x, np.float32)
    x16 = x.astype(np.float16)

    # weight stack [128, 2, 21, 256] fp16: dim1 = k-tile half
    W_all = np.zeros((NRW, D, D), np.float32)
    W_all[0:10] = np.asarray(W_fwd, np.float32)
    W_all[10:20] = np.asarray(W_rev, np.float32)
    W_all[20] = np.asarray(W_self, np.float32)
    wsb = np.zeros((128, 2, NRW, D), np.float16)
    for h in range(2):
        wsb[:, h, :, :] = W_all[:, h * 128:(h + 1) * 128, :].transpose(
            1, 0, 2).astype(np.float16)

    # bias table [21, 256] fp16
    ball = np.concatenate(
        [np.asarray(b_fwd, np.float32),
         np.asarray(b_rev, np.float32),
         np.asarray(b_self, np.float32)[None, :]], axis=0).astype(np.float16)

    nblk = NW * NRW
    nblk_pad = (nblk + GBC - 1) // GBC * GBC

    # ---- per-core edges keyed by (relW, local dst); dedupe cells
    core_key = [[] for _ in range(N_CORES)]
    core_src = [[] for _ in range(N_CORES)]
    for d in range(2):
        if d == 0:
            src_a, dst_a, relw_a = gov_idx, dep_idx, rel_idx
        else:
            src_a, dst_a, relw_a = dep_idx, gov_idx, rel_idx + 10
        core_of = dst_a // NODES_PER_CORE
        for c in range(N_CORES):
            m = core_of == c
            core_key[c].append(relw_a[m] * NODES_PER_CORE
                               + (dst_a[m] - c * NODES_PER_CORE))
            core_src[c].append(src_a[m])

    in_maps = []
    for c in range(N_CORES):
        key = np.concatenate(core_key[c])
        src = np.concatenate(core_src[c])
        order = np.argsort(key, kind="stable")
        key, src = key[order], src[order]
        ukey, start, cnt = np.unique(key, return_index=True,
                                     return_counts=True)
        single = cnt == 1
        multi = np.nonzero(~single)[0]
        comb_rows = np.zeros((len(multi), D), np.float32)
        for j, ui in enumerate(multi):
            s = start[ui]
            comb_rows[j] = x[src[s:s + cnt[ui]]].sum(0)
        gsrc = np.empty(ukey.shape[0], np.int64)
        gsrc[single] = src[start[single]]
        gsrc[~single] = N_NODES + np.arange(len(multi))
        relw = ukey // NODES_PER_CORE
        dstl = ukey % NODES_PER_CORE

        table = np.concatenate(
            [x16, comb_rows.astype(np.float16),
             np.zeros((1, D), np.float16)], axis=0)
        zrow = table.shape[0] - 1

        # block b = w*21 + r; column = dstl % 128
        src_all = np.full(nblk_pad * 128, zrow, np.int64)
        w_arr = dstl // 128
        pos = dstl % 128
        src_all[(w_arr * NRW + relw) * 128 + pos] = gsrc
        # self blocks: r = 20, every real dst
        dl = np.arange(NODES_PER_CORE)
        src_all[((dl // 128) * NRW + 20) * 128 + dl % 128] = \
            c * NODES_PER_CORE + dl

        # cnt table [21, NW*128] fp16: per-dst edge counts + const row
        cntb = np.zeros((NRW, NW * 128), np.float16)
        cntb[relw, w_arr * 128 + pos] = cnt.astype(np.float16)
        cntb[20, :NODES_PER_CORE] = 1.0

        # host gather + transpose into matmul-ready flat layout:
        # x_blocks[p, b*256 + j*128 + e] = feat (p + 128j) of col e of blk b
        A = table[src_all].reshape(nblk_pad, 128, 2, 128)   # [b, e, j, p]
        x_blocks = np.ascontiguousarray(
            A.transpose(3, 0, 2, 1)).reshape(128, nblk_pad * 256)

        in_maps.append({
            "x_blocks": x_blocks,
            "wsb": wsb,
            "ball": ball,
            "cntb": cntb,
        })

    return NW, nblk, nblk_pad, in_maps


# ---------------------------------------------------------------- device

def build_bass(nw, nblk, nblk_pad):
    nc = bacc.Bacc()
    x_blocks = nc.declare_dram_parameter("x_blocks", [128, nblk_pad * 256],
                                         F16, isOutput=False)
    wsb = nc.declare_dram_parameter("wsb", [128, 2, NRW, D], F16,
                                    isOutput=False)
    ball = nc.declare_dram_parameter("ball", [NRW, D], F16, isOutput=False)
    cntb = nc.declare_dram_parameter("cntb", [NRW, nw * 128], F16,
                                     isOutput=False)
    out = nc.declare_dram_parameter("out", [nw * 128, D], F16,
                                    isOutput=True)

    n_ch = nblk_pad // GBC
    no_mm = _os.environ.get("GCN_NO_MM") == "1"
    no_load = _os.environ.get("GCN_NO_LOAD") == "1"
    pe_only = _os.environ.get("GCN_PE_ONLY") == "1"

    with TileContext(nc) as tc:
        with (
            tc.tile_pool(name="cst", bufs=1) as cst,
            tc.tile_pool(name="xp", bufs=int(_os.environ.get("GCN_XPB", "8"))) as xp,
            tc.tile_pool(name="ot", bufs=4) as ot,
            tc.tile_pool(name="pm",
                         bufs=int(_os.environ.get("GCN_PMB", "8")),
                         space="PSUM") as pm,
        ):
            wsb_t = cst.tile([128, 2, NRW, D], F16, tag="wsb")
            nc.sync.dma_start(out=wsb_t[:], in_=wsb[:])
            ball_t = cst.tile([NRW, D], F16, tag="ball")
            nc.sync.dma_start(out=ball_t[:], in_=ball[:])
            cntb_t = cst.tile([NRW, nw * 128], F16, tag="cntb")
            nc.sync.dma_start(out=cntb_t[:], in_=cntb[:])

            chunks = [None] * n_ch

            def issue_load(j):
                if j >= n_ch or chunks[j] is not None:
                    return
                if pe_only:
                    # one resident chunk reused by every matmul
                    if chunks[0] is None:
                        ch = xp.tile([128, GBC * 256], F16, tag="x")
                        nc.sync.dma_start(out=ch[:], in_=x_blocks[:, 0:GBC * 256])
                        for jj in range(n_ch):
                            chunks[jj] = ch
                    return
                ch = xp.tile([128, GBC * 256], F16, tag="x")
                if no_load:
                    # tiny touch-write so downstream reads are legal
                    nc.sync.dma_start(out=ch[:, 0:256],
                                      in_=x_blocks[:, 0:256])
                else:
                    nc.sync.dma_start(
                        out=ch[:],
                        in_=x_blocks[:, j * GBC * 256:(j + 1) * GBC * 256])
                chunks[j] = ch

            reps = int(_os.environ.get("GCN_REPS", "1"))
            for _rep in range(reps):
                chunks[:] = [None] * n_ch
                issue_load(0)
                issue_load(1)
                issue_load(2)
                for w in range(nw):
                    if no_mm:
                        for r in range(NRW):
                            b = w * NRW + r
                            if b % GBC == 0:
                                issue_load(b // GBC + 3)
                        continue
                    ps = pm.tile([128, D], F32, tag="ps")
                    nc.tensor.matmul(
                        out=ps[:],
                        lhsT=cntb_t[:, w * 128:(w + 1) * 128],
                        rhs=ball_t[:],
                        start=True, stop=False)
                    for r in range(NRW):
                        b = w * NRW + r
                        if b % GBC == 0:
                            issue_load(b // GBC + 3)
                        ch = chunks[b // GBC]
                        s = (b % GBC) * 256
                        nc.tensor.matmul(
                            out=ps[:],
                            lhsT=ch[:, s:s + 128],
                            rhs=wsb_t[:, 0, r, :],
                            start=False, stop=False)
                        nc.tensor.matmul(
                            out=ps[:],
                            lhsT=ch[:, s + 128:s + 256],
                            rhs=wsb_t[:, 1, r, :],
                            start=False, stop=(r == NRW - 1))
                    o_t = ot.tile([128, D], F16, tag="o")
                    if w % 2 == 0:
                        nc.scalar.copy(out=o_t[:], in_=ps[:])
                    else:
                        nc.vector.tensor_copy(o_t[:], ps[:])
                    nc.sync.dma_start(out=out[w * 128:(w + 1) * 128, :],
                                      in_=o_t[:])
    nc.finalize()
    return nc


# ---------------------------------------------------------------- entry

def kernel(x, W_self, b_self, W_fwd, b_fwd, W_rev, b_rev,
           dep_idx, rel_idx, gov_idx, _trace=False, _trace_kwargs=None):
    nw, nblk, nblk_pad, in_maps = prepare(
        x, W_self, b_self, W_fwd, b_fwd, W_rev, b_rev,
        dep_idx, rel_idx, gov_idx)
    nc = build_bass(nw, nblk, nblk_pad)
    res = run_bass_kernel_spmd(nc, in_maps, list(range(N_CORES)),
                               trace=_trace, **(_trace_kwargs or {}))
    outs = [res.results[c]["out"][0:NODES_PER_CORE] for c in range(N_CORES)]
    kernel._last_results = res
    return np.concatenate(outs, axis=0).astype(np.float32)


# revision 22
# speedup vs baseline: 1.0533x; 1.0533x over previous
"""Dependency-GCN via host pre-gather + per-window PSUM accumulation
for 8 Trainium2 NeuronCores.  No scatter, no SWDGE, no collectives.

Strategy (single SPMD program):
  - Each core owns a contiguous range of 3750 destination nodes; edges
    are routed to their dst-owner core (fwd: dep, rev: gov).
  - Host pre-combines edges sharing (direction, relation, dst): their
    source rows are summed on the host, so each (direction, relation)
    group has at most ONE cell per dst.
  - Destinations are grouped into 30 windows of 128.  For window w and
    relation-weight r (20 edge rels + self as rel 20), a 128-column
    lhsT block holds the cell source features at column = dst % 128
    (zero columns where the (r, dst) cell is absent).  The 21 rel
    blocks of a window accumulate into ONE PSUM tile via matmul
    accumulation -- the "scatter" happens positionally in PSUM.
  - Bias rides as a k=21 matmul per window: lhsT = per-dst edge counts
    for each rel (+ const-1 row), rhs = [b_fwd; b_rev; b_self].  This
    reproduces out += cnt_r * b_r exactly (multi-edge cells carry
    their edge count).
  - The gather is done ON THE HOST: x_blocks [128, nblk*256] fp16
    holds, for block b, the transposed source features laid out as
    (k-partition, b*256 + k_half*128 + column) so plain contiguous
    DMA loads (multi-KB descriptor runs) yield ready-to-use matmul
    lhsT tiles.
  - Per window: 43 fp16 matmuls (FWL hides weight loads) -> one
    PSUM->SBUF fp32->fp16 copy (alternating Activation/DVE) -> one
    plain contiguous DMA write of the finished 128 output rows.
"""

import sys

if "/opt/trn_rl_repo" not in sys.path:
    sys.path.insert(0, "/opt/trn_rl_repo")

import os as _os
import numpy as np

import concourse.bacc as bacc
import concourse.mybir as mybir
from concourse.tile import TileContext
from concourse.bass_utils import run_bass_kernel_spmd

F32 = mybir.dt.float32
F16 = mybir.dt.float16

N_NODES = 30000
N_REL = 10
D = 256
N_CORES = 8
NODES_PER_CORE = N_NODES // N_CORES          # 3750
NW = (NODES_PER_CORE + 127) // 128            # 30 windows of 128 dsts
NRW = 21                                      # 20 edge rels + self
GBC = int(_os.environ.get("GCN_GBC", "8"))   # blocks per load chunk


# ---------------------------------------------------------------- host prep

def prepare(x, W_self, b_self, W_fwd, b_fwd, W_rev, b_rev,
            dep_idx, rel_idx, gov_idx):
    dep_idx = np.asarray(dep_idx).astype(np.int64)
    rel_idx = np.asarray(rel_idx).astype(np.int64)
    gov_idx = np.asarray(gov_idx).astype(np.int64)
    x = np.asarray(x, np.float32)
    x16 = x.astype(np.float16)

    # weight stack [128, 2, 21, 256] fp16: dim1 = k-tile half
    W_all = np.zeros((NRW, D, D), np.float32)
    W_all[0:10] = np.asarray(W_fwd, np.float32)
    W_all[10:20] = np.asarray(W_rev, np.float32)
    W_all[20] = np.asarray(W_self, np.float32)
    wsb = np.zeros((128, 2, NRW, D), np.float16)
    for h in range(2):
        wsb[:, h, :, :] = W_all[:, h * 128:(h + 1) * 128, :].transpose(
            1, 0, 2).astype(np.float16)

    # bias table [21, 256] fp16
    ball = np.concatenate(
        [np.asarray(b_fwd, np.float32),
         np.asarray(b_rev, np.float32),
         np.asarray(b_self, np.float32)[None, :]], axis=0).astype(np.float16)

    nblk = NW * NRW
    nblk_pad = (nblk + GBC - 1) // GBC * GBC

    # ---- per-core edges keyed by (relW, local dst); dedupe cells
    core_key = [[] for _ in range(N_CORES)]
    core_src = [[] for _ in range(N_CORES)]
    for d in range(2):
        if d == 0:
            src_a, dst_a, relw_a = gov_idx, dep_idx, rel_idx
        else:
            src_a, dst_a, relw_a = dep_idx, gov_idx, rel_idx + 10
        core_of = dst_a // NODES_PER_CORE
        for c in range(N_CORES):
            m = core_of == c
            core_key[c].append(relw_a[m] * NODES_PER_CORE
                               + (dst_a[m] - c * NODES_PER_CORE))
            core_src[c].append(src_a[m])

    in_maps = []
    for c in range(N_CORES):
        key = np.concatenate(core_key[c])
        src = np.concatenate(core_src[c])
        order = np.argsort(key, kind="stable")
        key, src = key[order], src[order]
        ukey, start, cnt = np.unique(key, return_index=True,
                                     return_counts=True)
        single = cnt == 1
        multi = np.nonzero(~single)[0]
        comb_rows = np.zeros((len(multi), D), np.float32)
        for j, ui in enumerate(multi):
            s = start[ui]
            comb_rows[j] = x[src[s:s + cnt[ui]]].sum(0)
        gsrc = np.empty(ukey.shape[0], np.int64)
        gsrc[single] = src[start[single]]
        gsrc[~single] = N_NODES + np.arange(len(multi))
        relw = ukey // NODES_PER_CORE
        dstl = ukey % NODES_PER_CORE

        table = np.concatenate(
            [x16, comb_rows.astype(np.float16),
             np.zeros((1, D), np.float16)], axis=0)
        zrow = table.shape[0] - 1

        # block b = w*21 + r; column = dstl % 128
        src_all = np.full(nblk_pad * 128, zrow, np.int64)
        w_arr = dstl // 128
        pos = dstl % 128
        src_all[(w_arr * NRW + relw) * 128 + pos] = gsrc
        # self blocks: r = 20, every real dst
        dl = np.arange(NODES_PER_CORE)
        src_all[((dl // 128) * NRW + 20) * 128 + dl % 128] = \
            c * NODES_PER_CORE + dl

        # cnt table [21, NW*128] fp16: per-dst edge counts + const row
        cntb = np.zeros((NRW, NW * 128), np.float16)
        cntb[relw, w_arr * 128 + pos] = cnt.astype(np.float16)
        cntb[20, :NODES_PER_CORE] = 1.0

        # host gather + transpose into matmul-ready flat layout:
        # x_blocks[p, b*256 + j*128 + e] = feat (p + 128j) of col e of blk b
        A = table[src_all].reshape(nblk_pad, 128, 2, 128)   # [b, e, j, p]
        x_blocks = np.ascontiguousarray(
            A.transpose(3, 0, 2, 1)).reshape(128, nblk_pad * 256)

        in_maps.append({
            "x_blocks": x_blocks,
            "wsb": wsb,
            "ball": ball,
            "cntb": cntb,
        })

    return NW, nblk, nblk_pad, in_maps


# ---------------------------------------------------------------- device

def build_bass(nw, nblk, nblk_pad):
    nc = bacc.Bacc()
    x_blocks = nc.declare_dram_parameter("x_blocks", [128, nblk_pad * 256],
                                         F16, isOutput=False)
    wsb = nc.declare_dram_parameter("wsb", [128, 2, NRW, D], F16,
                                    isOutput=False)
    ball = nc.declare_dram_parameter("ball", [NRW, D], F16, isOutput=False)
    cntb = nc.declare_dram_parameter("cntb", [NRW, nw * 128], F16,
                                     isOutput=False)
    out = nc.declare_dram_parameter("out", [nw * 128, D], F16,
                                    isOutput=True)

    n_ch = nblk_pad // GBC

    with TileContext(nc) as tc:
        with (
            tc.tile_pool(name="cst", bufs=1) as cst,
            tc.tile_pool(name="xp", bufs=int(_os.environ.get("GCN_XPB", "8"))) as xp,
            tc.tile_pool(name="ot", bufs=4) as ot,
            tc.tile_pool(name="pm",
                         bufs=int(_os.environ.get("GCN_PMB", "8")),
                         space="PSUM") as pm,
        ):
            wsb_t = cst.tile([128, 2, NRW, D], F16, tag="wsb")
            nc.sync.dma_start(out=wsb_t[:], in_=wsb[:])
            ball_t = cst.tile([NRW, D], F16, tag="ball")
            nc.sync.dma_start(out=ball_t[:], in_=ball[:])
            cntb_t = cst.tile([NRW, nw * 128], F16, tag="cntb")
            nc.sync.dma_start(out=cntb_t[:], in_=cntb[:])

            chunks = [None] * n_ch

            def issue_load(j):
                if j >= n_ch or chunks[j] is not None:
                    return
                ch = xp.tile([128, GBC * 256], F16, tag="x")
                nc.sync.dma_start(
                    out=ch[:],
                    in_=x_blocks[:, j * GBC * 256:(j + 1) * GBC * 256])
                chunks[j] = ch

            reps = int(_os.environ.get("GCN_REPS", "1"))
            for _rep in range(reps):
                chunks[:] = [None] * n_ch
                issue_load(0)
                issue_load(1)
                issue_load(2)
                for w in range(nw):
                    ps = pm.tile([128, D], F32, tag="ps")
                    nc.tensor.matmul(
                        out=ps[:],
                        lhsT=cntb_t[:, w * 128:(w + 1) * 128],
                        rhs=ball_t[:],
                        start=True, stop=False)
                    for r in range(NRW):
                        b = w * NRW + r
                        if b % GBC == 0:
                            issue_load(b // GBC + 3)
                        ch = chunks[b // GBC]
                        s = (b % GBC) * 256
                        nc.tensor.matmul(
                            out=ps[:],
                            lhsT=ch[:, s:s + 128],
                            rhs=wsb_t[:, 0, r, :],
                            start=False, stop=False)
                        nc.tensor.matmul(
                            out=ps[:],
                            lhsT=ch[:, s + 128:s + 256],
                            rhs=wsb_t[:, 1, r, :],
                            start=False, stop=(r == NRW - 1))
                    o_t = ot.tile([128, D], F16, tag="o")
                    if w % 2 == 0:
                        nc.scalar.copy(out=o_t[:], in_=ps[:])
                    else:
                        nc.vector.tensor_copy(o_t[:], ps[:])
                    nc.sync.dma_start(out=out[w * 128:(w + 1) * 128, :],
                                      in_=o_t[:])
    nc.finalize()
    return nc


# ---------------------------------------------------------------- entry

def kernel(x, W_self, b_self, W_fwd, b_fwd, W_rev, b_rev,
           dep_idx, rel_idx, gov_idx, _trace=False, _trace_kwargs=None):
    nw, nblk, nblk_pad, in_maps = prepare(
        x, W_self, b_self, W_fwd, b_fwd, W_rev, b_rev,
        dep_idx, rel_idx, gov_idx)
    nc = build_bass(nw, nblk, nblk_pad)
    res = run_bass_kernel_spmd(nc, in_maps, list(range(N_CORES)),
                               trace=_trace, **(_trace_kwargs or {}))
    outs = [res.results[c]["out"][0:NODES_PER_CORE] for c in range(N_CORES)]
    kernel._last_results = res
    return np.concatenate(outs, axis=0).astype(np.float32)
